# revision 11
# baseline (speedup 1.0000x reference)
"""Trainium2 Bass kernel for pointer-generator style attention.

Math (per batch b, seq s, hidden h; B=16, S=4096, H=1024):
    dec_fea = s_t_hat @ W_dec.T + b_dec                      # [B,H]
    att     = ef[b,s,:] + dec_fea[b,:] + cov[b,s]*W_c        # [B,S,H]
    scores  = tanh(att) @ v                                   # [B,S]
    attn    = renorm(softmax(scores) * mask)                  # [B,S]
    c_t     = attn @ encoder_outputs                          # [B,H]
returns (c_t, attn, attn, scores)

Sharding: data-parallel over batch, 2 batches per core on 8 cores.

Per-core pipeline (DMA-bound target ~210us at ~330GB/s):
  - PE computes only the rank-2 addend 1(x)dec_fea + cov(x)W_c into PSUM
    (float32r, full rate) and the attn.eo matvec for c_t.
  - DVE: one fused scalar_tensor_tensor (ef + psum_addend) and one fused
    (tanh_out * v) with accum_out -> scores column. 2 ops per [128,1024].
  - ACT: tanh only.
  - encoder tensors stream as 2MB DMAs of [128, 4x1024] (4 seq-subtiles).
"""

import numpy as np

import concourse.bacc as bacc
import concourse.bass as bass
import concourse.mybir as mybir
import concourse.tile as tile
from concourse.bass_utils import run_bass_kernel_spmd

B, S, H = 16, 4096, 1024
NCORES = 8
BL = B // NCORES            # batches per core (2)
P = 128                     # partitions
TPB = S // P                # seq tiles per batch (32)
NT = BL * TPB               # seq tiles per core (64)
ROWS = BL * S               # rows per core (8192)
KH = H // P                 # hidden chunks of 128 (8)
MMF = 512                   # matmul moving free-dim chunk (fp32 max)
CSUB = 4                    # seq subtiles per DMA load (2MB transfers)

F32 = mybir.dt.float32
F32R = mybir.dt.float32r
AF = mybir.ActivationFunctionType
OP = mybir.AluOpType


def build_program(n_cores=NCORES, bl=BL, s_len=S):
    """Build the per-core Bass program (SPMD: same program, different data)."""
    tpb = s_len // P
    nt = bl * tpb
    rows = bl * s_len
    nblk = nt // CSUB           # big DMA blocks per core

    nc = bacc.Bacc("TRN2", target_bir_lowering=False, debug=False,
                   num_devices=n_cores)

    ef_d = nc.dram_tensor("ef", [rows, H], F32, kind="ExternalInput").ap()
    eo_d = nc.dram_tensor("eo", [rows, H], F32R, kind="ExternalInput").ap()
    srep_d = nc.dram_tensor("srep", [bl, P, H], F32, kind="ExternalInput").ap()
    r2lc_d = nc.dram_tensor("r2lc", [2, rows], F32R, kind="ExternalInput").ap()
    maskc_d = nc.dram_tensor("maskc", [P, nt], F32, kind="ExternalInput").ap()
    wdec_d = nc.dram_tensor("wdec", [H, H], F32, kind="ExternalInput").ap()
    wc_d = nc.dram_tensor("wc", [1, H], F32R, kind="ExternalInput").ap()
    bdecc_d = nc.dram_tensor("bdecc", [P, KH], F32, kind="ExternalInput").ap()
    vrep_d = nc.dram_tensor("vrep", [P, H], F32, kind="ExternalInput").ap()
    ident_d = nc.dram_tensor("ident", [P, P], F32, kind="ExternalInput").ap()

    ct_d = nc.dram_tensor("ct_out", [bl, H], F32, kind="ExternalOutput").ap()
    attn_d = nc.dram_tensor("attn_out", [bl, s_len], F32, kind="ExternalOutput").ap()
    sc_d = nc.dram_tensor("scores_out", [bl, s_len], F32, kind="ExternalOutput").ap()

    # big-block views: block n covers seq tiles [n*CSUB, (n+1)*CSUB)
    ef_blk = ef_d.rearrange("(n c p) h -> n p c h", c=CSUB, p=P)
    eo_blk = eo_d.rearrange("(n c p) h -> n p c h", c=CSUB, p=P)
    wdec_blk = wdec_d.rearrange("(n c p) h -> n p c h", c=CSUB, p=P)

    with tile.TileContext(nc) as tc:
        with (
            tc.tile_pool(name="persist", bufs=1) as persist,
            tc.tile_pool(name="efp", bufs=5) as efp,
            tc.tile_pool(name="eop", bufs=2) as eop,
            tc.tile_pool(name="scr", bufs=2) as scr,
            tc.tile_pool(name="psA", bufs=2, space="PSUM") as psA,
            tc.tile_pool(name="psC", bufs=1, space="PSUM") as psC,
        ):
            # ---------------- persistent tiles / constants ----------------
            v_sb = persist.tile([P, H], F32)
            nc.sync.dma_start(v_sb[:], vrep_d[:])
            ident_sb = persist.tile([P, P], F32)
            nc.sync.dma_start(ident_sb[:], ident_d[:])
            maskc_sb = persist.tile([P, nt], F32)
            nc.sync.dma_start(maskc_sb[:], maskc_d[:])
            bdecc_sb = persist.tile([P, KH], F32)
            nc.sync.dma_start(bdecc_sb[:], bdecc_d[:])
            onesc = persist.tile([P, 1], F32)
            nc.vector.memset(onesc[:], 1.0)
            onesr = persist.tile([1, P], F32)
            nc.vector.memset(onesr[:], 1.0)

            srep_sb = []
            for b in range(bl):
                t = persist.tile([P, H], F32, name=f"srep_sb{b}")
                nc.sync.dma_start(t[:], srep_d[b])
                srep_sb.append(t)

            # rank-2 lhsT per tile: [2,128] slices; row 0 = ones, row 1 = cov.
            r2l = persist.tile([2, rows], F32R)
            nc.sync.dma_start(r2l[:], r2lc_d[:])
            r2l_v = r2l.rearrange("k (n p) -> k n p", p=P)   # [2, nt, 128]

            # rank-2 rhs per batch: row 0 = dec_fea_b, row 1 = W_c.
            r2r = []
            for b in range(bl):
                t = persist.tile([2, H], F32R, name=f"r2r{b}")
                nc.sync.dma_start(t[1:2, :], wc_d[:])
                r2r.append(t)

            scores_c = persist.tile([P, nt], F32)
            p_c = persist.tile([P, nt], F32)
            pm_c = persist.tile([P, nt], F32)
            attn_c = persist.tile([P, nt], F32)
            attn_r = persist.tile([P, nt], F32R)

            # ---------------- dec_fea = s_t_hat @ W_dec.T + b_dec ----------
            # per-partition (n) dots via fused multiply+reduce on DVE.
            dec_cols = persist.tile([P, bl * KH], F32)
            for wb in range(KH // CSUB):
                wt = efp.tile([P, CSUB * H], F32, name="wt", tag="ef")
                nc.sync.dma_start(wt[:], wdec_blk[wb])
                for jc in range(CSUB):
                    j = wb * CSUB + jc
                    wt_j = wt[:, jc * H:(jc + 1) * H]
                    for b in range(bl):
                        ttr_s = scr.tile([P, 1], F32, name="ttr_s", tag="scr")
                        nc.vector.scalar_tensor_tensor(
                            ttr_s.broadcast_to(wt_j.shape), wt_j, 1.0,
                            srep_sb[b][:], op0=OP.mult, op1=OP.mult,
                            accum_out=dec_cols[:, b * KH + j:b * KH + j + 1])
            for b in range(bl):
                nc.vector.tensor_add(dec_cols[:, b * KH:(b + 1) * KH],
                                     dec_cols[:, b * KH:(b + 1) * KH],
                                     bdecc_sb[:, :])
            # scatter dec_fea chunks (partition layout) into r2r row 0 (free
            # layout) via staging row + tiny DMAs + f32r round-trip copy.
            for b in range(bl):
                dec_row = persist.tile([1, H], F32, name=f"dec_row{b}")
                for j in range(KH):
                    nc.gpsimd.dma_start(dec_row[0:1, j * P:(j + 1) * P],
                                        dec_cols[:, b * KH + j:b * KH + j + 1])
                nc.vector.tensor_copy(r2r[b][0:1, :], dec_row[:])

            for b in range(bl):
                # ---------------- stage A: scores ----------------
                for blk in range(tpb // CSUB):
                    n = b * (tpb // CSUB) + blk
                    ef_sb = efp.tile([P, CSUB * H], F32, name="ef_sb", tag="ef")
                    nc.sync.dma_start(ef_sb[:], ef_blk[n])
                    for cidx in range(CSUB):
                        g = n * CSUB + cidx
                        ef_sub = ef_sb[:, cidx * H:(cidx + 1) * H]
                        ps = psA.tile([P, H], F32, name="ps")
                        for c in range(0, H, MMF):
                            nc.tensor.matmul(ps[:, c:c + MMF],
                                             r2l_v[:, g, :],
                                             r2r[b][:, c:c + MMF],
                                             start=True, stop=True)
                        # pre-activation: ef + (dec + cov*W_c), in place
                        nc.vector.scalar_tensor_tensor(
                            ef_sub, ef_sub, 1.0, ps[:],
                            op0=OP.mult, op1=OP.add)
                        nc.scalar.activation(ef_sub, ef_sub, AF.Tanh)
                        ev_s = scr.tile([P, 1], F32, name="ev_s", tag="scr")
                        nc.vector.scalar_tensor_tensor(
                            ev_s.broadcast_to(ef_sub.shape), ef_sub, 1.0,
                            v_sb[:], op0=OP.mult, op1=OP.mult,
                            accum_out=scores_c[:, g:g + 1])

                # ---------------- softmax over s (batch b) ----------------
                sl = slice(b * tpb, (b + 1) * tpb)
                # no max-subtraction: |scores| <= ||v||_1 ~ 26, exp safe
                nc.scalar.activation(p_c[:, sl], scores_c[:, sl], AF.Exp)
                nc.vector.tensor_mul(pm_c[:, sl], p_c[:, sl], maskc_sb[:, sl])
                cs = psC.tile([1, tpb], F32, name="cs", tag="sm")
                nc.tensor.matmul(cs[0:1, :], onesc[:], pm_c[:, sl])
                ssum = persist.tile([1, 1], F32, name=f"ssum{b}")
                nc.vector.reduce_sum(ssum[:], cs[0:1, :],
                                     axis=mybir.AxisListType.X)
                rec = persist.tile([1, 1], F32, name=f"rec{b}")
                nc.vector.reciprocal(rec[:], ssum[:])
                bc = psC.tile([P, 1], F32, name="bc", tag="sm")
                nc.tensor.matmul(bc[:, 0:1], onesr[:], rec[0:1, 0:1])
                bc_sb = persist.tile([P, 1], F32, name=f"bc_sb{b}")
                nc.vector.tensor_copy(bc_sb[:], bc[:, 0:1])
                nc.vector.tensor_scalar_mul(attn_c[:, sl], pm_c[:, sl],
                                            bc_sb[:, 0:1])
                nc.vector.tensor_scalar_mul(attn_r[:, sl], pm_c[:, sl],
                                            bc_sb[:, 0:1])

                # ---- scores/attn rows for this batch (overlaps stage C)
                bsl = slice(b * tpb, (b + 1) * tpb)
                for nm, cols, dram in (("sc", scores_c, sc_d),
                                       ("at", attn_c, attn_d)):
                    tp = psC.tile([tpb, P], F32, name=f"tp{nm}{b}", tag="sm")
                    nc.tensor.transpose(tp[:, :], cols[:, bsl], ident_sb[:])
                    tsb = persist.tile([tpb, P], F32, name=f"tsb{nm}{b}")
                    nc.scalar.copy(tsb[:], tp[:, :])
                    nc.gpsimd.dma_start(
                        dram[b:b + 1, :].rearrange("one (n p) -> (one n) p", p=P),
                        tsb[:])

                # ---------------- stage C: c_t ----------------
                ct_ps = psC.tile([1, H], F32, name="ct_ps", tag="ct")
                for blk in range(tpb // CSUB):
                    n = b * (tpb // CSUB) + blk
                    eo_sb = eop.tile([P, CSUB * H], F32R, name="eo_sb", tag="eo")
                    nc.sync.dma_start(eo_sb[:], eo_blk[n])
                    for cidx in range(CSUB):
                        g = n * CSUB + cidx
                        t = blk * CSUB + cidx
                        eo_sub = eo_sb[:, cidx * H:(cidx + 1) * H]
                        for c in range(0, H, MMF):
                            nc.tensor.matmul(ct_ps[0:1, c:c + MMF],
                                             attn_r[:, g:g + 1],
                                             eo_sub[:, c:c + MMF],
                                             start=(t == 0),
                                             stop=(t == tpb - 1))
                ct_sb = persist.tile([1, H], F32, name=f"ct_sb{b}")
                nc.scalar.copy(ct_sb[:], ct_ps[0:1, :])
                nc.gpsimd.dma_start(ct_d[b:b + 1, :], ct_sb[:])

    nc.compile()
    return nc


def make_in_maps(s_t_hat, encoder_outputs, encoder_feature, enc_padding_mask,
                 coverage, W_dec, b_dec, W_c, v, n_cores=NCORES, bl=BL, s_len=S):
    tpb = s_len // P
    nt = bl * tpb
    rows = bl * s_len
    b_total = n_cores * bl

    ef_full = np.ascontiguousarray(encoder_feature, np.float32).reshape(
        b_total, s_len, H)
    eo_full = np.ascontiguousarray(encoder_outputs, np.float32)
    s_full = np.ascontiguousarray(s_t_hat, np.float32)
    mask_full = np.ascontiguousarray(enc_padding_mask, np.float32)
    cov_full = np.ascontiguousarray(coverage, np.float32)

    wdec = np.ascontiguousarray(W_dec, np.float32)
    wc = np.ascontiguousarray(W_c, np.float32).reshape(1, H)
    bdecc = np.ascontiguousarray(
        np.asarray(b_dec, np.float32).reshape(KH, P).T)          # [P, KH]
    vrep = np.ascontiguousarray(
        np.broadcast_to(np.asarray(v, np.float32)[None, :], (P, H)))
    ident = np.eye(P, dtype=np.float32)

    in_maps = []
    for i in range(n_cores):
        b0 = i * bl
        maskc = np.ascontiguousarray(
            mask_full[b0:b0 + bl].reshape(bl, tpb, P)
            .transpose(2, 0, 1).reshape(P, nt))
        in_maps.append({
            "ef": np.ascontiguousarray(ef_full[b0:b0 + bl].reshape(rows, H)),
            "eo": np.ascontiguousarray(eo_full[b0:b0 + bl].reshape(rows, H)),
            "srep": np.ascontiguousarray(
                np.broadcast_to(s_full[b0:b0 + bl, None, :], (bl, P, H))),
            "r2lc": np.ascontiguousarray(np.stack([
                np.ones(rows, np.float32),
                cov_full[b0:b0 + bl].reshape(-1)])),
            "maskc": maskc,
            "wdec": wdec,
            "wc": wc,
            "bdecc": bdecc,
            "vrep": vrep,
            "ident": ident,
        })
    return in_maps


def _gather(results, n_cores=NCORES):
    ct = np.concatenate([results[i]["ct_out"] for i in range(n_cores)], 0)
    attn = np.concatenate([results[i]["attn_out"] for i in range(n_cores)], 0)
    scores = np.concatenate([results[i]["scores_out"] for i in range(n_cores)], 0)
    return ct, attn, attn, scores


_NC_CACHE = {}


def _get_program():
    if "nc" not in _NC_CACHE:
        _NC_CACHE["nc"] = build_program()
    return _NC_CACHE["nc"]


def run(trace=False, tmpdir=None, **inputs):
    nc = _get_program()
    in_maps = make_in_maps(**inputs)
    res = run_bass_kernel_spmd(nc, in_maps, list(range(NCORES)), trace=trace,
                               tmpdir=tmpdir)
    return _gather(res.results), res


def kernel(**inputs):
    out, _ = run(trace=False, **inputs)
    return out


# revision 14
# speedup vs baseline: 1.0051x; 1.0051x over previous
"""Trainium2 Bass kernel for pointer-generator style attention.

Math (per batch b, seq s, hidden h; B=16, S=4096, H=1024):
    dec_fea = s_t_hat @ W_dec.T + b_dec                      # [B,H]
    att     = ef[b,s,:] + dec_fea[b,:] + cov[b,s]*W_c        # [B,S,H]
    scores  = tanh(att) @ v                                   # [B,S]
    attn    = renorm(softmax(scores) * mask)                  # [B,S]
    c_t     = attn @ encoder_outputs                          # [B,H]
returns (c_t, attn, attn, scores)

Sharding: data-parallel over batch, 2 batches per core on 8 cores.

Per-core pipeline (DMA-bound target ~210us at ~330GB/s):
  - PE computes only the rank-2 addend 1(x)dec_fea + cov(x)W_c into PSUM
    (float32r, full rate) and the attn.eo matvec for c_t.
  - DVE: one fused scalar_tensor_tensor (ef + psum_addend) and one fused
    (tanh_out * v) with accum_out -> scores column. 2 ops per [128,1024].
  - ACT: tanh only.
  - encoder tensors stream as 2MB DMAs of [128, 4x1024] (4 seq-subtiles).
"""

import numpy as np

import concourse.bacc as bacc
import concourse.bass as bass
import concourse.mybir as mybir
import concourse.tile as tile
from concourse.bass_utils import run_bass_kernel_spmd

B, S, H = 16, 4096, 1024
NCORES = 8
BL = B // NCORES            # batches per core (2)
P = 128                     # partitions
TPB = S // P                # seq tiles per batch (32)
NT = BL * TPB               # seq tiles per core (64)
ROWS = BL * S               # rows per core (8192)
KH = H // P                 # hidden chunks of 128 (8)
MMF = 512                   # matmul moving free-dim chunk (fp32 max)
CSUB = 4                    # seq subtiles per DMA load (2MB transfers)
ZSPL = 8                    # trailing subtiles/batch whose matvec runs on DVE

F32 = mybir.dt.float32
F32R = mybir.dt.float32r
AF = mybir.ActivationFunctionType
OP = mybir.AluOpType


def build_program(n_cores=NCORES, bl=BL, s_len=S):
    """Build the per-core Bass program (SPMD: same program, different data)."""
    tpb = s_len // P
    nt = bl * tpb
    rows = bl * s_len
    nblk = nt // CSUB           # big DMA blocks per core

    nc = bacc.Bacc("TRN2", target_bir_lowering=False, debug=False,
                   num_devices=n_cores)

    ef_d = nc.dram_tensor("ef", [rows, H], F32, kind="ExternalInput").ap()
    eo_d = nc.dram_tensor("eo", [rows, H], F32R, kind="ExternalInput").ap()
    srep_d = nc.dram_tensor("srep", [bl, P, H], F32, kind="ExternalInput").ap()
    r2lc_d = nc.dram_tensor("r2lc", [2, rows], F32R, kind="ExternalInput").ap()
    maskc_d = nc.dram_tensor("maskc", [P, nt], F32, kind="ExternalInput").ap()
    wdec_d = nc.dram_tensor("wdec", [H, H], F32, kind="ExternalInput").ap()
    wc_d = nc.dram_tensor("wc", [1, H], F32R, kind="ExternalInput").ap()
    bdecc_d = nc.dram_tensor("bdecc", [P, KH], F32, kind="ExternalInput").ap()
    vrep_d = nc.dram_tensor("vrep", [P, H], F32, kind="ExternalInput").ap()
    ident_d = nc.dram_tensor("ident", [P, P], F32, kind="ExternalInput").ap()

    ct_d = nc.dram_tensor("ct_out", [bl, H], F32, kind="ExternalOutput").ap()
    attn_d = nc.dram_tensor("attn_out", [bl, s_len], F32, kind="ExternalOutput").ap()
    sc_d = nc.dram_tensor("scores_out", [bl, s_len], F32, kind="ExternalOutput").ap()

    # big-block views: block n covers seq tiles [n*CSUB, (n+1)*CSUB)
    ef_blk = ef_d.rearrange("(n c p) h -> n p c h", c=CSUB, p=P)
    eo_blk = eo_d.rearrange("(n c p) h -> n p c h", c=CSUB, p=P)
    wdec_blk = wdec_d.rearrange("(n c p) h -> n p c h", c=CSUB, p=P)

    with tile.TileContext(nc) as tc:
        with (
            tc.tile_pool(name="persist", bufs=1) as persist,
            tc.tile_pool(name="efp", bufs=5) as efp,
            tc.tile_pool(name="eop", bufs=3) as eop,
            tc.tile_pool(name="scr", bufs=2) as scr,
            tc.tile_pool(name="psA", bufs=2, space="PSUM") as psA,
            tc.tile_pool(name="psC", bufs=1, space="PSUM") as psC,
        ):
            # ---------------- persistent tiles / constants ----------------
            v_sb = persist.tile([P, H], F32)
            nc.sync.dma_start(v_sb[:], vrep_d[:])
            ident_sb = persist.tile([P, P], F32)
            nc.sync.dma_start(ident_sb[:], ident_d[:])
            maskc_sb = persist.tile([P, nt], F32)
            nc.sync.dma_start(maskc_sb[:], maskc_d[:])
            bdecc_sb = persist.tile([P, KH], F32)
            nc.sync.dma_start(bdecc_sb[:], bdecc_d[:])
            onesc = persist.tile([P, 1], F32)
            nc.vector.memset(onesc[:], 1.0)
            onesr = persist.tile([1, P], F32)
            nc.vector.memset(onesr[:], 1.0)

            srep_sb = []
            for b in range(bl):
                t = efp.tile([P, CSUB * H], F32, name=f"srep_sb{b}", tag="ef")
                nc.sync.dma_start(t[:, 0:H], srep_d[b])
                srep_sb.append(t[:, 0:H])
            ctacc = persist.tile([P, H], F32)

            # rank-2 lhsT per tile: [2,128] slices; row 0 = ones, row 1 = cov.
            r2l = persist.tile([2, rows], F32R)
            nc.sync.dma_start(r2l[:], r2lc_d[:])
            r2l_v = r2l.rearrange("k (n p) -> k n p", p=P)   # [2, nt, 128]

            # rank-2 rhs per batch: row 0 = dec_fea_b, row 1 = W_c.
            r2r = []
            for b in range(bl):
                t = persist.tile([2, H], F32R, name=f"r2r{b}")
                nc.sync.dma_start(t[1:2, :], wc_d[:])
                r2r.append(t)

            scores_c = persist.tile([P, nt], F32)
            p_c = persist.tile([P, nt], F32)
            pm_c = persist.tile([P, nt], F32)
            attn_c = persist.tile([P, nt], F32)
            attn_r = persist.tile([P, nt], F32R)

            # ---------------- dec_fea = s_t_hat @ W_dec.T + b_dec ----------
            # per-partition (n) dots via fused multiply+reduce on DVE.
            dec_cols = persist.tile([P, bl * KH], F32)
            for wb in range(KH // CSUB):
                wt = efp.tile([P, CSUB * H], F32, name="wt", tag="ef")
                nc.sync.dma_start(wt[:], wdec_blk[wb])
                for jc in range(CSUB):
                    j = wb * CSUB + jc
                    wt_j = wt[:, jc * H:(jc + 1) * H]
                    for b in range(bl):
                        ttr_s = scr.tile([P, 1], F32, name="ttr_s", tag="scr")
                        nc.vector.scalar_tensor_tensor(
                            ttr_s.broadcast_to(wt_j.shape), wt_j, 1.0,
                            srep_sb[b][:], op0=OP.mult, op1=OP.mult,
                            accum_out=dec_cols[:, b * KH + j:b * KH + j + 1])
            for b in range(bl):
                nc.vector.tensor_add(dec_cols[:, b * KH:(b + 1) * KH],
                                     dec_cols[:, b * KH:(b + 1) * KH],
                                     bdecc_sb[:, :])
            # scatter dec_fea chunks (partition layout) into r2r row 0 (free
            # layout) via staging row + tiny DMAs + f32r round-trip copy.
            for b in range(bl):
                dec_row = persist.tile([1, H], F32, name=f"dec_row{b}")
                for j in range(KH):
                    nc.gpsimd.dma_start(dec_row[0:1, j * P:(j + 1) * P],
                                        dec_cols[:, b * KH + j:b * KH + j + 1])
                nc.vector.tensor_copy(r2r[b][0:1, :], dec_row[:])

            for b in range(bl):
                # ---------------- stage A: scores ----------------
                for blk in range(tpb // CSUB):
                    n = b * (tpb // CSUB) + blk
                    ef_sb = efp.tile([P, CSUB * H], F32, name="ef_sb", tag="ef")
                    nc.sync.dma_start(ef_sb[:], ef_blk[n])
                    for cidx in range(CSUB):
                        g = n * CSUB + cidx
                        ef_sub = ef_sb[:, cidx * H:(cidx + 1) * H]
                        ps = psA.tile([P, H], F32, name="ps")
                        for c in range(0, H, MMF):
                            nc.tensor.matmul(ps[:, c:c + MMF],
                                             r2l_v[:, g, :],
                                             r2r[b][:, c:c + MMF],
                                             start=True, stop=True)
                        # pre-activation: ef + (dec + cov*W_c), in place
                        nc.vector.scalar_tensor_tensor(
                            ef_sub, ef_sub, 1.0, ps[:],
                            op0=OP.mult, op1=OP.add)
                        nc.scalar.activation(ef_sub, ef_sub, AF.Tanh)
                        ev_s = scr.tile([P, 1], F32, name="ev_s", tag="scr")
                        nc.vector.scalar_tensor_tensor(
                            ev_s.broadcast_to(ef_sub.shape), ef_sub, 1.0,
                            v_sb[:], op0=OP.mult, op1=OP.mult,
                            accum_out=scores_c[:, g:g + 1])

                # ---------------- softmax over s (batch b) ----------------
                sl = slice(b * tpb, (b + 1) * tpb)
                # no max-subtraction: |scores| <= ||v||_1 ~ 26, exp safe
                nc.scalar.activation(p_c[:, sl], scores_c[:, sl], AF.Exp)
                nc.vector.tensor_mul(pm_c[:, sl], p_c[:, sl], maskc_sb[:, sl])
                cs = psC.tile([1, tpb], F32, name="cs", tag="sm")
                nc.tensor.matmul(cs[0:1, :], onesc[:], pm_c[:, sl])
                ssum = persist.tile([1, 1], F32, name=f"ssum{b}")
                nc.vector.reduce_sum(ssum[:], cs[0:1, :],
                                     axis=mybir.AxisListType.X)
                rec = persist.tile([1, 1], F32, name=f"rec{b}")
                nc.vector.reciprocal(rec[:], ssum[:])
                bc = psC.tile([P, 1], F32, name="bc", tag="sm")
                nc.tensor.matmul(bc[:, 0:1], onesr[:], rec[0:1, 0:1])
                bc_sb = persist.tile([P, 1], F32, name=f"bc_sb{b}")
                nc.vector.tensor_copy(bc_sb[:], bc[:, 0:1])
                nc.vector.tensor_scalar_mul(attn_c[:, sl], pm_c[:, sl],
                                            bc_sb[:, 0:1])
                nc.vector.tensor_scalar_mul(attn_r[:, sl], pm_c[:, sl],
                                            bc_sb[:, 0:1])

                # ---- scores/attn rows for this batch (overlaps stage C)
                bsl = slice(b * tpb, (b + 1) * tpb)
                for nm, cols, dram in (("sc", scores_c, sc_d),
                                       ("at", attn_c, attn_d)):
                    tp = psC.tile([tpb, P], F32, name=f"tp{nm}{b}", tag="sm")
                    nc.tensor.transpose(tp[:, :], cols[:, bsl], ident_sb[:])
                    tsb = persist.tile([tpb, P], F32, name=f"tsb{nm}{b}")
                    nc.scalar.copy(tsb[:], tp[:, :])
                    nc.gpsimd.dma_start(
                        dram[b:b + 1, :].rearrange("one (n p) -> (one n) p", p=P),
                        tsb[:])

                # ---------------- stage C: c_t ----------------
                zspl = min(ZSPL, tpb)
                ct_ps = psC.tile([1, H], F32, name="ct_ps", tag="ct")
                for blk in range(tpb // CSUB):
                    n = b * (tpb // CSUB) + blk
                    eo_sb = eop.tile([P, CSUB * H], F32R, name="eo_sb", tag="eo")
                    nc.sync.dma_start(eo_sb[:], eo_blk[n])
                    for cidx in range(CSUB):
                        g = n * CSUB + cidx
                        t = blk * CSUB + cidx
                        eo_sub = eo_sb[:, cidx * H:(cidx + 1) * H]
                        if t < tpb - zspl:
                            for c in range(0, H, MMF):
                                nc.tensor.matmul(ct_ps[0:1, c:c + MMF],
                                                 attn_r[:, g:g + 1],
                                                 eo_sub[:, c:c + MMF],
                                                 start=(t == 0), stop=False)
                        elif t == tpb - zspl:
                            nc.vector.tensor_scalar_mul(
                                ctacc[:], eo_sub.bitcast(F32),
                                attn_c[:, g:g + 1])
                        else:
                            nc.vector.scalar_tensor_tensor(
                                ctacc[:], eo_sub.bitcast(F32),
                                attn_c[:, g:g + 1], ctacc[:],
                                op0=OP.mult, op1=OP.add)
                # fold the DVE partial (sum over partitions) into ct_ps
                for c in range(0, H, MMF):
                    nc.tensor.matmul(ct_ps[0:1, c:c + MMF], onesc[:],
                                     ctacc[:, c:c + MMF],
                                     start=(zspl == tpb), stop=True)
                ct_sb = persist.tile([1, H], F32, name=f"ct_sb{b}")
                nc.scalar.copy(ct_sb[:], ct_ps[0:1, :])
                nc.gpsimd.dma_start(ct_d[b:b + 1, :], ct_sb[:])

    nc.compile()
    return nc


def make_in_maps(s_t_hat, encoder_outputs, encoder_feature, enc_padding_mask,
                 coverage, W_dec, b_dec, W_c, v, n_cores=NCORES, bl=BL, s_len=S):
    tpb = s_len // P
    nt = bl * tpb
    rows = bl * s_len
    b_total = n_cores * bl

    ef_full = np.ascontiguousarray(encoder_feature, np.float32).reshape(
        b_total, s_len, H)
    eo_full = np.ascontiguousarray(encoder_outputs, np.float32)
    s_full = np.ascontiguousarray(s_t_hat, np.float32)
    mask_full = np.ascontiguousarray(enc_padding_mask, np.float32)
    cov_full = np.ascontiguousarray(coverage, np.float32)

    wdec = np.ascontiguousarray(W_dec, np.float32)
    wc = np.ascontiguousarray(W_c, np.float32).reshape(1, H)
    bdecc = np.ascontiguousarray(
        np.asarray(b_dec, np.float32).reshape(KH, P).T)          # [P, KH]
    vrep = np.ascontiguousarray(
        np.broadcast_to(np.asarray(v, np.float32)[None, :], (P, H)))
    ident = np.eye(P, dtype=np.float32)

    in_maps = []
    for i in range(n_cores):
        b0 = i * bl
        maskc = np.ascontiguousarray(
            mask_full[b0:b0 + bl].reshape(bl, tpb, P)
            .transpose(2, 0, 1).reshape(P, nt))
        in_maps.append({
            "ef": np.ascontiguousarray(ef_full[b0:b0 + bl].reshape(rows, H)),
            "eo": np.ascontiguousarray(eo_full[b0:b0 + bl].reshape(rows, H)),
            "srep": np.ascontiguousarray(
                np.broadcast_to(s_full[b0:b0 + bl, None, :], (bl, P, H))),
            "r2lc": np.ascontiguousarray(np.stack([
                np.ones(rows, np.float32),
                cov_full[b0:b0 + bl].reshape(-1)])),
            "maskc": maskc,
            "wdec": wdec,
            "wc": wc,
            "bdecc": bdecc,
            "vrep": vrep,
            "ident": ident,
        })
    return in_maps


def _gather(results, n_cores=NCORES):
    ct = np.concatenate([results[i]["ct_out"] for i in range(n_cores)], 0)
    attn = np.concatenate([results[i]["attn_out"] for i in range(n_cores)], 0)
    scores = np.concatenate([results[i]["scores_out"] for i in range(n_cores)], 0)
    return ct, attn, attn, scores


_NC_CACHE = {}


def _get_program():
    if "nc" not in _NC_CACHE:
        _NC_CACHE["nc"] = build_program()
    return _NC_CACHE["nc"]


def run(trace=False, tmpdir=None, **inputs):
    nc = _get_program()
    in_maps = make_in_maps(**inputs)
    res = run_bass_kernel_spmd(nc, in_maps, list(range(NCORES)), trace=trace,
                               tmpdir=tmpdir)
    return _gather(res.results), res


def kernel(**inputs):
    out, _ = run(trace=False, **inputs)
    return out


# revision 15
# speedup vs baseline: 1.0917x; 1.0862x over previous
"""Trainium2 Bass kernel for pointer-generator style attention.

Math (per batch b, seq s, hidden h; B=16, S=4096, H=1024):
    dec_fea = s_t_hat @ W_dec.T + b_dec                      # [B,H]
    att     = ef[b,s,:] + dec_fea[b,:] + cov[b,s]*W_c        # [B,S,H]
    scores  = tanh(att) @ v                                   # [B,S]
    attn    = renorm(softmax(scores) * mask)                  # [B,S]
    c_t     = attn @ encoder_outputs                          # [B,H]
returns (c_t, attn, attn, scores)

Sharding: data-parallel over batch, 2 batches per core on 8 cores.

Per-core pipeline (DMA-bound target ~210us at ~330GB/s):
  - PE computes only the rank-2 addend 1(x)dec_fea + cov(x)W_c into PSUM
    (float32r, full rate) and the attn.eo matvec for c_t.
  - DVE: one fused scalar_tensor_tensor (ef + psum_addend) and one fused
    (tanh_out * v) with accum_out -> scores column. 2 ops per [128,1024].
  - ACT: tanh only.
  - encoder tensors stream as 2MB DMAs of [128, 4x1024] (4 seq-subtiles).
"""

import numpy as np

import concourse.bacc as bacc
import concourse.bass as bass
import concourse.mybir as mybir
import concourse.tile as tile
from concourse.bass_utils import run_bass_kernel_spmd

B, S, H = 16, 4096, 1024
NCORES = 8
BL = B // NCORES            # batches per core (2)
P = 128                     # partitions
TPB = S // P                # seq tiles per batch (32)
NT = BL * TPB               # seq tiles per core (64)
ROWS = BL * S               # rows per core (8192)
KH = H // P                 # hidden chunks of 128 (8)
MMF = 512                   # matmul moving free-dim chunk (fp32 max)
CSUB = 4                    # seq subtiles per DMA load (2MB transfers)
ZSPL = 8                    # trailing subtiles/batch whose matvec runs on DVE

F32 = mybir.dt.float32
F32R = mybir.dt.float32r
AF = mybir.ActivationFunctionType
OP = mybir.AluOpType


def build_program(n_cores=NCORES, bl=BL, s_len=S):
    """Build the per-core Bass program (SPMD: same program, different data)."""
    tpb = s_len // P
    nt = bl * tpb
    rows = bl * s_len
    nblk = nt // CSUB           # big DMA blocks per core

    nc = bacc.Bacc("TRN2", target_bir_lowering=False, debug=False,
                   num_devices=n_cores)

    ef_d = nc.dram_tensor("ef", [rows, H], F32, kind="ExternalInput").ap()
    eo_d = nc.dram_tensor("eo", [rows, H], F32R, kind="ExternalInput").ap()
    srep_d = nc.dram_tensor("srep", [bl, P, H], F32, kind="ExternalInput").ap()
    r2lc_d = nc.dram_tensor("r2lc", [2, rows], F32R, kind="ExternalInput").ap()
    maskc_d = nc.dram_tensor("maskc", [P, nt], F32, kind="ExternalInput").ap()
    wdec_d = nc.dram_tensor("wdec", [H, H], F32, kind="ExternalInput").ap()
    wc_d = nc.dram_tensor("wc", [1, H], F32R, kind="ExternalInput").ap()
    bdecc_d = nc.dram_tensor("bdecc", [P, KH], F32, kind="ExternalInput").ap()
    vrep_d = nc.dram_tensor("vrep", [P, H], F32, kind="ExternalInput").ap()
    ident_d = nc.dram_tensor("ident", [P, P], F32, kind="ExternalInput").ap()

    ct_d = nc.dram_tensor("ct_out", [bl, H], F32, kind="ExternalOutput").ap()
    attn_d = nc.dram_tensor("attn_out", [bl, s_len], F32, kind="ExternalOutput").ap()
    sc_d = nc.dram_tensor("scores_out", [bl, s_len], F32, kind="ExternalOutput").ap()

    # big-block views: block n covers seq tiles [n*CSUB, (n+1)*CSUB)
    ef_blk = ef_d.rearrange("(n c p) h -> n p c h", c=CSUB, p=P)
    eo_blk = eo_d.rearrange("(n c p) h -> n p c h", c=CSUB, p=P)
    wdec_blk = wdec_d.rearrange("(n c p) h -> n p c h", c=CSUB, p=P)

    with tile.TileContext(nc) as tc:
        with (
            tc.tile_pool(name="persist", bufs=1) as persist,
            tc.tile_pool(name="efp", bufs=5) as efp,
            tc.tile_pool(name="eop", bufs=3) as eop,
            tc.tile_pool(name="scr", bufs=2) as scr,
            tc.tile_pool(name="psA", bufs=2, space="PSUM") as psA,
            tc.tile_pool(name="psC", bufs=1, space="PSUM") as psC,
        ):
            # ---------------- persistent tiles / constants ----------------
            v_sb = persist.tile([P, H], F32)
            nc.sync.dma_start(v_sb[:], vrep_d[:])
            ident_sb = persist.tile([P, P], F32)
            nc.sync.dma_start(ident_sb[:], ident_d[:])
            maskc_sb = persist.tile([P, nt], F32)
            nc.sync.dma_start(maskc_sb[:], maskc_d[:])
            bdecc_sb = persist.tile([P, KH], F32)
            nc.sync.dma_start(bdecc_sb[:], bdecc_d[:])
            onesc = persist.tile([P, 1], F32)
            nc.vector.memset(onesc[:], 1.0)
            onesr = persist.tile([1, P], F32)
            nc.vector.memset(onesr[:], 1.0)

            srep_sb = []
            for b in range(bl):
                t = efp.tile([P, CSUB * H], F32, name=f"srep_sb{b}", tag="ef")
                nc.sync.dma_start(t[:, 0:H], srep_d[b])
                srep_sb.append(t[:, 0:H])
            ctacc = persist.tile([P, H], F32)

            # rank-2 lhsT per tile: [2,128] slices; row 0 = ones, row 1 = cov.
            r2l = persist.tile([2, rows], F32R)
            nc.sync.dma_start(r2l[:], r2lc_d[:])
            r2l_v = r2l.rearrange("k (n p) -> k n p", p=P)   # [2, nt, 128]

            # rank-2 rhs per batch: row 0 = dec_fea_b, row 1 = W_c.
            r2r = []
            for b in range(bl):
                t = persist.tile([2, H], F32R, name=f"r2r{b}")
                nc.sync.dma_start(t[1:2, :], wc_d[:])
                r2r.append(t)

            scores_c = persist.tile([P, nt], F32)
            p_c = persist.tile([P, nt], F32)
            pm_c = persist.tile([P, nt], F32)
            attn_c = persist.tile([P, nt], F32)
            attn_r = persist.tile([P, nt], F32R)

            # ---------------- dec_fea = s_t_hat @ W_dec.T + b_dec ----------
            # per-partition (n) dots via fused multiply+reduce on DVE.
            dec_cols = persist.tile([P, bl * KH], F32)
            wts = []
            for wb in range(KH // CSUB):
                wt = efp.tile([P, CSUB * H], F32, name="wt", tag="ef")
                nc.sync.dma_start(wt[:], wdec_blk[wb])
                wts.append(wt)
            for b in range(bl):
                for wb in range(KH // CSUB):
                    for jc in range(CSUB):
                        j = wb * CSUB + jc
                        wt_j = wts[wb][:, jc * H:(jc + 1) * H]
                        ttr_s = scr.tile([P, 1], F32, name="ttr_s", tag="scr")
                        nc.vector.scalar_tensor_tensor(
                            ttr_s.broadcast_to(wt_j.shape), wt_j, 1.0,
                            srep_sb[b][:], op0=OP.mult, op1=OP.mult,
                            accum_out=dec_cols[:, b * KH + j:b * KH + j + 1])
                nc.vector.tensor_add(dec_cols[:, b * KH:(b + 1) * KH],
                                     dec_cols[:, b * KH:(b + 1) * KH],
                                     bdecc_sb[:, :])
                # scatter dec chunks (partition layout) into r2r row 0
                dec_row = persist.tile([1, H], F32, name=f"dec_row{b}")
                for j in range(KH):
                    nc.sync.dma_start(dec_row[0:1, j * P:(j + 1) * P],
                                      dec_cols[:, b * KH + j:b * KH + j + 1])
                nc.vector.tensor_copy(r2r[b][0:1, :], dec_row[:])

            for b in range(bl):
                # ---------------- stage A: scores ----------------
                for blk in range(tpb // CSUB):
                    n = b * (tpb // CSUB) + blk
                    ef_sb = efp.tile([P, CSUB * H], F32, name="ef_sb", tag="ef")
                    nc.sync.dma_start(ef_sb[:], ef_blk[n])
                    for cidx in range(CSUB):
                        g = n * CSUB + cidx
                        ef_sub = ef_sb[:, cidx * H:(cidx + 1) * H]
                        ps = psA.tile([P, H], F32, name="ps")
                        for c in range(0, H, MMF):
                            nc.tensor.matmul(ps[:, c:c + MMF],
                                             r2l_v[:, g, :],
                                             r2r[b][:, c:c + MMF],
                                             start=True, stop=True)
                        # pre-activation: ef + (dec + cov*W_c), in place
                        nc.vector.scalar_tensor_tensor(
                            ef_sub, ef_sub, 1.0, ps[:],
                            op0=OP.mult, op1=OP.add)
                        nc.scalar.activation(ef_sub, ef_sub, AF.Tanh)
                        ev_s = scr.tile([P, 1], F32, name="ev_s", tag="scr")
                        nc.vector.scalar_tensor_tensor(
                            ev_s.broadcast_to(ef_sub.shape), ef_sub, 1.0,
                            v_sb[:], op0=OP.mult, op1=OP.mult,
                            accum_out=scores_c[:, g:g + 1])

                # ---------------- softmax over s (batch b) ----------------
                sl = slice(b * tpb, (b + 1) * tpb)
                # no max-subtraction: |scores| <= ||v||_1 ~ 26, exp safe
                nc.scalar.activation(p_c[:, sl], scores_c[:, sl], AF.Exp)
                nc.vector.tensor_mul(pm_c[:, sl], p_c[:, sl], maskc_sb[:, sl])
                cs = psC.tile([1, tpb], F32, name="cs", tag="sm")
                nc.tensor.matmul(cs[0:1, :], onesc[:], pm_c[:, sl])
                ssum = persist.tile([1, 1], F32, name=f"ssum{b}")
                nc.vector.reduce_sum(ssum[:], cs[0:1, :],
                                     axis=mybir.AxisListType.X)
                rec = persist.tile([1, 1], F32, name=f"rec{b}")
                nc.vector.reciprocal(rec[:], ssum[:])
                bc = psC.tile([P, 1], F32, name="bc", tag="sm")
                nc.tensor.matmul(bc[:, 0:1], onesr[:], rec[0:1, 0:1])
                bc_sb = persist.tile([P, 1], F32, name=f"bc_sb{b}")
                nc.vector.tensor_copy(bc_sb[:], bc[:, 0:1])
                nc.vector.tensor_scalar_mul(attn_c[:, sl], pm_c[:, sl],
                                            bc_sb[:, 0:1])
                nc.vector.tensor_scalar_mul(attn_r[:, sl], pm_c[:, sl],
                                            bc_sb[:, 0:1])

                # ---- scores/attn rows for this batch (overlaps stage C)
                bsl = slice(b * tpb, (b + 1) * tpb)
                for nm, cols, dram in (("sc", scores_c, sc_d),
                                       ("at", attn_c, attn_d)):
                    tp = psC.tile([tpb, P], F32, name=f"tp{nm}{b}", tag="sm")
                    nc.tensor.transpose(tp[:, :], cols[:, bsl], ident_sb[:])
                    tsb = persist.tile([tpb, P], F32, name=f"tsb{nm}{b}")
                    nc.scalar.copy(tsb[:], tp[:, :])
                    nc.gpsimd.dma_start(
                        dram[b:b + 1, :].rearrange("one (n p) -> (one n) p", p=P),
                        tsb[:])

                # ---------------- stage C: c_t ----------------
                zspl = min(ZSPL, tpb)
                ct_ps = psC.tile([1, H], F32, name="ct_ps", tag="ct")
                for blk in range(tpb // CSUB):
                    n = b * (tpb // CSUB) + blk
                    eo_sb = eop.tile([P, CSUB * H], F32R, name="eo_sb", tag="eo")
                    nc.sync.dma_start(eo_sb[:], eo_blk[n])
                    for cidx in range(CSUB):
                        g = n * CSUB + cidx
                        t = blk * CSUB + cidx
                        eo_sub = eo_sb[:, cidx * H:(cidx + 1) * H]
                        if t == 0:
                            nc.vector.tensor_scalar_mul(
                                ctacc[:], eo_sub.bitcast(F32),
                                attn_c[:, g:g + 1])
                        elif t < zspl:
                            nc.vector.scalar_tensor_tensor(
                                ctacc[:], eo_sub.bitcast(F32),
                                attn_c[:, g:g + 1], ctacc[:],
                                op0=OP.mult, op1=OP.add)
                        else:
                            for c in range(0, H, MMF):
                                nc.tensor.matmul(ct_ps[0:1, c:c + MMF],
                                                 attn_r[:, g:g + 1],
                                                 eo_sub[:, c:c + MMF],
                                                 start=(t == zspl), stop=False)
                # fold the DVE partial (sum over partitions) into ct_ps
                for c in range(0, H, MMF):
                    nc.tensor.matmul(ct_ps[0:1, c:c + MMF], onesc[:],
                                     ctacc[:, c:c + MMF],
                                     start=(zspl == tpb), stop=True)
                ct_sb = persist.tile([1, H], F32, name=f"ct_sb{b}")
                nc.scalar.copy(ct_sb[:], ct_ps[0:1, :])
                nc.gpsimd.dma_start(ct_d[b:b + 1, :], ct_sb[:])

    nc.compile()
    return nc


def make_in_maps(s_t_hat, encoder_outputs, encoder_feature, enc_padding_mask,
                 coverage, W_dec, b_dec, W_c, v, n_cores=NCORES, bl=BL, s_len=S):
    tpb = s_len // P
    nt = bl * tpb
    rows = bl * s_len
    b_total = n_cores * bl

    ef_full = np.ascontiguousarray(encoder_feature, np.float32).reshape(
        b_total, s_len, H)
    eo_full = np.ascontiguousarray(encoder_outputs, np.float32)
    s_full = np.ascontiguousarray(s_t_hat, np.float32)
    mask_full = np.ascontiguousarray(enc_padding_mask, np.float32)
    cov_full = np.ascontiguousarray(coverage, np.float32)

    wdec = np.ascontiguousarray(W_dec, np.float32)
    wc = np.ascontiguousarray(W_c, np.float32).reshape(1, H)
    bdecc = np.ascontiguousarray(
        np.asarray(b_dec, np.float32).reshape(KH, P).T)          # [P, KH]
    vrep = np.ascontiguousarray(
        np.broadcast_to(np.asarray(v, np.float32)[None, :], (P, H)))
    ident = np.eye(P, dtype=np.float32)

    in_maps = []
    for i in range(n_cores):
        b0 = i * bl
        maskc = np.ascontiguousarray(
            mask_full[b0:b0 + bl].reshape(bl, tpb, P)
            .transpose(2, 0, 1).reshape(P, nt))
        in_maps.append({
            "ef": np.ascontiguousarray(ef_full[b0:b0 + bl].reshape(rows, H)),
            "eo": np.ascontiguousarray(eo_full[b0:b0 + bl].reshape(rows, H)),
            "srep": np.ascontiguousarray(
                np.broadcast_to(s_full[b0:b0 + bl, None, :], (bl, P, H))),
            "r2lc": np.ascontiguousarray(np.stack([
                np.ones(rows, np.float32),
                cov_full[b0:b0 + bl].reshape(-1)])),
            "maskc": maskc,
            "wdec": wdec,
            "wc": wc,
            "bdecc": bdecc,
            "vrep": vrep,
            "ident": ident,
        })
    return in_maps


def _gather(results, n_cores=NCORES):
    ct = np.concatenate([results[i]["ct_out"] for i in range(n_cores)], 0)
    attn = np.concatenate([results[i]["attn_out"] for i in range(n_cores)], 0)
    scores = np.concatenate([results[i]["scores_out"] for i in range(n_cores)], 0)
    return ct, attn, attn, scores


_NC_CACHE = {}


def _get_program():
    if "nc" not in _NC_CACHE:
        _NC_CACHE["nc"] = build_program()
    return _NC_CACHE["nc"]


def run(trace=False, tmpdir=None, **inputs):
    nc = _get_program()
    in_maps = make_in_maps(**inputs)
    res = run_bass_kernel_spmd(nc, in_maps, list(range(NCORES)), trace=trace,
                               tmpdir=tmpdir)
    return _gather(res.results), res


def kernel(**inputs):
    out, _ = run(trace=False, **inputs)
    return out


# revision 16
# speedup vs baseline: 1.0920x; 1.0003x over previous
"""Trainium2 Bass kernel for pointer-generator style attention.

Math (per batch b, seq s, hidden h; B=16, S=4096, H=1024):
    dec_fea = s_t_hat @ W_dec.T + b_dec                      # [B,H]
    att     = ef[b,s,:] + dec_fea[b,:] + cov[b,s]*W_c        # [B,S,H]
    scores  = tanh(att) @ v                                   # [B,S]
    attn    = renorm(softmax(scores) * mask)                  # [B,S]
    c_t     = attn @ encoder_outputs                          # [B,H]
returns (c_t, attn, attn, scores)

Sharding: data-parallel over batch, 2 batches per core on 8 cores.

Per-core pipeline (DMA-bound target ~210us at ~330GB/s):
  - PE computes only the rank-2 addend 1(x)dec_fea + cov(x)W_c into PSUM
    (float32r, full rate) and the attn.eo matvec for c_t.
  - DVE: one fused scalar_tensor_tensor (ef + psum_addend) and one fused
    (tanh_out * v) with accum_out -> scores column. 2 ops per [128,1024].
  - ACT: tanh only.
  - encoder tensors stream as 2MB DMAs of [128, 4x1024] (4 seq-subtiles).
"""

import numpy as np

import concourse.bacc as bacc
import concourse.bass as bass
import concourse.mybir as mybir
import concourse.tile as tile
from concourse.bass_utils import run_bass_kernel_spmd

B, S, H = 16, 4096, 1024
NCORES = 8
BL = B // NCORES            # batches per core (2)
P = 128                     # partitions
TPB = S // P                # seq tiles per batch (32)
NT = BL * TPB               # seq tiles per core (64)
ROWS = BL * S               # rows per core (8192)
KH = H // P                 # hidden chunks of 128 (8)
MMF = 512                   # matmul moving free-dim chunk (fp32 max)
CSUB = 4                    # seq subtiles per DMA load (2MB transfers)
ZSPL = 8                    # trailing subtiles/batch whose matvec runs on DVE

F32 = mybir.dt.float32
F32R = mybir.dt.float32r
AF = mybir.ActivationFunctionType
OP = mybir.AluOpType


def build_program(n_cores=NCORES, bl=BL, s_len=S):
    """Build the per-core Bass program (SPMD: same program, different data)."""
    tpb = s_len // P
    nt = bl * tpb
    rows = bl * s_len
    nblk = nt // CSUB           # big DMA blocks per core

    nc = bacc.Bacc("TRN2", target_bir_lowering=False, debug=False,
                   num_devices=n_cores)

    ef_d = nc.dram_tensor("ef", [rows, H], F32, kind="ExternalInput").ap()
    eo_d = nc.dram_tensor("eo", [rows, H], F32R, kind="ExternalInput").ap()
    srep_d = nc.dram_tensor("srep", [bl, P, H], F32, kind="ExternalInput").ap()
    r2lc_d = nc.dram_tensor("r2lc", [2, rows], F32R, kind="ExternalInput").ap()
    maskc_d = nc.dram_tensor("maskc", [P, nt], F32, kind="ExternalInput").ap()
    wdec_d = nc.dram_tensor("wdec", [H, H], F32, kind="ExternalInput").ap()
    wc_d = nc.dram_tensor("wc", [1, H], F32R, kind="ExternalInput").ap()
    bdecc_d = nc.dram_tensor("bdecc", [P, KH], F32, kind="ExternalInput").ap()
    vrep_d = nc.dram_tensor("vrep", [P, H], F32, kind="ExternalInput").ap()
    ident_d = nc.dram_tensor("ident", [P, P], F32, kind="ExternalInput").ap()

    ct_d = nc.dram_tensor("ct_out", [bl, H], F32, kind="ExternalOutput").ap()
    attn_d = nc.dram_tensor("attn_out", [bl, s_len], F32, kind="ExternalOutput").ap()
    sc_d = nc.dram_tensor("scores_out", [bl, s_len], F32, kind="ExternalOutput").ap()

    # big-block views: block n covers seq tiles [n*CSUB, (n+1)*CSUB)
    ef_blk = ef_d.rearrange("(n c p) h -> n p c h", c=CSUB, p=P)
    eo_blk = eo_d.rearrange("(n c p) h -> n p c h", c=CSUB, p=P)
    wdec_blk = wdec_d.rearrange("(n c p) h -> n p c h", c=CSUB, p=P)

    with tile.TileContext(nc) as tc:
        with (
            tc.tile_pool(name="persist", bufs=1) as persist,
            tc.tile_pool(name="efp", bufs=5) as efp,
            tc.tile_pool(name="eop", bufs=3) as eop,
            tc.tile_pool(name="scr", bufs=2) as scr,
            tc.tile_pool(name="psA", bufs=2, space="PSUM") as psA,
            tc.tile_pool(name="psC", bufs=1, space="PSUM") as psC,
        ):
            # ---------------- persistent tiles / constants ----------------
            v_sb = persist.tile([P, H], F32)
            nc.sync.dma_start(v_sb[:], vrep_d[:])
            ident_sb = persist.tile([P, P], F32)
            nc.sync.dma_start(ident_sb[:], ident_d[:])
            maskc_sb = persist.tile([P, nt], F32)
            nc.sync.dma_start(maskc_sb[:], maskc_d[:])
            bdecc_sb = persist.tile([P, KH], F32)
            nc.sync.dma_start(bdecc_sb[:], bdecc_d[:])
            onesc = persist.tile([P, 1], F32)
            nc.vector.memset(onesc[:], 1.0)
            onesr = persist.tile([1, P], F32)
            nc.vector.memset(onesr[:], 1.0)

            srep_sb = []
            for b in range(bl):
                t = efp.tile([P, CSUB * H], F32, name=f"srep_sb{b}", tag="ef")
                nc.sync.dma_start(t[:, 0:H], srep_d[b])
                srep_sb.append(t[:, 0:H])
            ctacc = persist.tile([P, H], F32)

            # rank-2 lhsT per tile: [2,128] slices; row 0 = ones, row 1 = cov.
            r2l = persist.tile([2, rows], F32R)
            nc.sync.dma_start(r2l[:], r2lc_d[:])
            r2l_v = r2l.rearrange("k (n p) -> k n p", p=P)   # [2, nt, 128]

            # rank-2 rhs per batch: row 0 = dec_fea_b, row 1 = W_c.
            r2r = []
            for b in range(bl):
                t = persist.tile([2, H], F32R, name=f"r2r{b}")
                nc.sync.dma_start(t[1:2, :], wc_d[:])
                r2r.append(t)

            scores_c = persist.tile([P, nt], F32)
            p_c = persist.tile([P, nt], F32)
            pm_c = persist.tile([P, nt], F32)
            attn_c = persist.tile([P, nt], F32)
            attn_r = persist.tile([P, nt], F32R)

            # ---------------- dec_fea = s_t_hat @ W_dec.T + b_dec ----------
            # per-partition (n) dots via fused multiply+reduce on DVE.
            dec_cols = persist.tile([P, bl * KH], F32)
            wts = []
            for wb in range(KH // CSUB):
                wt = efp.tile([P, CSUB * H], F32, name="wt", tag="ef")
                nc.sync.dma_start(wt[:], wdec_blk[wb])
                wts.append(wt)
            for b in range(bl):
                for wb in range(KH // CSUB):
                    for jc in range(CSUB):
                        j = wb * CSUB + jc
                        wt_j = wts[wb][:, jc * H:(jc + 1) * H]
                        ttr_s = scr.tile([P, 1], F32, name="ttr_s", tag="scr")
                        nc.vector.scalar_tensor_tensor(
                            ttr_s.broadcast_to(wt_j.shape), wt_j, 1.0,
                            srep_sb[b][:], op0=OP.mult, op1=OP.mult,
                            accum_out=dec_cols[:, b * KH + j:b * KH + j + 1])
                nc.vector.tensor_add(dec_cols[:, b * KH:(b + 1) * KH],
                                     dec_cols[:, b * KH:(b + 1) * KH],
                                     bdecc_sb[:, :])
                # scatter dec chunks (partition layout) into r2r row 0
                dec_row = persist.tile([1, H], F32, name=f"dec_row{b}")
                for j in range(KH):
                    nc.sync.dma_start(dec_row[0:1, j * P:(j + 1) * P],
                                      dec_cols[:, b * KH + j:b * KH + j + 1])
                nc.vector.tensor_copy(r2r[b][0:1, :], dec_row[:])

            zspl = min(ZSPL, tpb)
            nblk_b = tpb // CSUB
            ct_state = {}

            def stage_a_block(b, blk):
                n = b * nblk_b + blk
                ef_sb = efp.tile([P, CSUB * H], F32, name="ef_sb", tag="ef")
                nc.sync.dma_start(ef_sb[:], ef_blk[n])
                for cidx in range(CSUB):
                    g = n * CSUB + cidx
                    ef_sub = ef_sb[:, cidx * H:(cidx + 1) * H]
                    ps = psA.tile([P, H], F32, name="ps")
                    for c in range(0, H, MMF):
                        nc.tensor.matmul(ps[:, c:c + MMF],
                                         r2l_v[:, g, :],
                                         r2r[b][:, c:c + MMF],
                                         start=True, stop=True)
                    # pre-activation: ef + (dec + cov*W_c), in place
                    nc.vector.scalar_tensor_tensor(
                        ef_sub, ef_sub, 1.0, ps[:],
                        op0=OP.mult, op1=OP.add)
                    nc.scalar.activation(ef_sub, ef_sub, AF.Tanh)
                    ev_s = scr.tile([P, 1], F32, name="ev_s", tag="scr")
                    nc.vector.scalar_tensor_tensor(
                        ev_s.broadcast_to(ef_sub.shape), ef_sub, 1.0,
                        v_sb[:], op0=OP.mult, op1=OP.mult,
                        accum_out=scores_c[:, g:g + 1])

            def softmax_b(b):
                sl = slice(b * tpb, (b + 1) * tpb)
                # no max-subtraction: |scores| <= ||v||_1 ~ 26, exp safe
                nc.scalar.activation(p_c[:, sl], scores_c[:, sl], AF.Exp)
                nc.vector.tensor_mul(pm_c[:, sl], p_c[:, sl], maskc_sb[:, sl])
                cs = psC.tile([1, tpb], F32, name="cs", tag="sm")
                nc.tensor.matmul(cs[0:1, :], onesc[:], pm_c[:, sl])
                ssum = persist.tile([1, 1], F32, name=f"ssum{b}")
                nc.vector.reduce_sum(ssum[:], cs[0:1, :],
                                     axis=mybir.AxisListType.X)
                rec = persist.tile([1, 1], F32, name=f"rec{b}")
                nc.vector.reciprocal(rec[:], ssum[:])
                bc = psC.tile([P, 1], F32, name="bc", tag="sm")
                nc.tensor.matmul(bc[:, 0:1], onesr[:], rec[0:1, 0:1])
                bc_sb = persist.tile([P, 1], F32, name=f"bc_sb{b}")
                nc.vector.tensor_copy(bc_sb[:], bc[:, 0:1])
                nc.vector.tensor_scalar_mul(attn_c[:, sl], pm_c[:, sl],
                                            bc_sb[:, 0:1])
                nc.vector.tensor_scalar_mul(attn_r[:, sl], pm_c[:, sl],
                                            bc_sb[:, 0:1])
                # scores/attn row-layout outputs (overlap downstream work)
                for nm, cols, dram in (("sc", scores_c, sc_d),
                                       ("at", attn_c, attn_d)):
                    tp = psC.tile([tpb, P], F32, name=f"tp{nm}{b}", tag="sm")
                    nc.tensor.transpose(tp[:, :], cols[:, sl], ident_sb[:])
                    tsb = persist.tile([tpb, P], F32, name=f"tsb{nm}{b}")
                    nc.scalar.copy(tsb[:], tp[:, :])
                    nc.gpsimd.dma_start(
                        dram[b:b + 1, :].rearrange("one (n p) -> (one n) p", p=P),
                        tsb[:])

            def stage_c_block(b, blk):
                if blk == 0:
                    ct_state[b] = psC.tile([1, H], F32, name=f"ct_ps{b}",
                                           tag="ct")
                ct_ps = ct_state[b]
                n = b * nblk_b + blk
                eo_sb = eop.tile([P, CSUB * H], F32R, name="eo_sb", tag="eo")
                nc.sync.dma_start(eo_sb[:], eo_blk[n])
                for cidx in range(CSUB):
                    g = n * CSUB + cidx
                    t = blk * CSUB + cidx
                    eo_sub = eo_sb[:, cidx * H:(cidx + 1) * H]
                    if t == 0:
                        nc.vector.tensor_scalar_mul(
                            ctacc[:], eo_sub.bitcast(F32),
                            attn_c[:, g:g + 1])
                    elif t < zspl:
                        nc.vector.scalar_tensor_tensor(
                            ctacc[:], eo_sub.bitcast(F32),
                            attn_c[:, g:g + 1], ctacc[:],
                            op0=OP.mult, op1=OP.add)
                    else:
                        for c in range(0, H, MMF):
                            nc.tensor.matmul(ct_ps[0:1, c:c + MMF],
                                             attn_r[:, g:g + 1],
                                             eo_sub[:, c:c + MMF],
                                             start=(t == zspl), stop=False)

            def stage_c_finish(b):
                ct_ps = ct_state[b]
                # fold the DVE partial (sum over partitions) into ct_ps
                for c in range(0, H, MMF):
                    nc.tensor.matmul(ct_ps[0:1, c:c + MMF], onesc[:],
                                     ctacc[:, c:c + MMF],
                                     start=(zspl == tpb), stop=True)
                ct_sb = persist.tile([1, H], F32, name=f"ct_sb{b}")
                nc.scalar.copy(ct_sb[:], ct_ps[0:1, :])
                nc.gpsimd.dma_start(ct_d[b:b + 1, :], ct_sb[:])

            # schedule: A(0) | softmax(0) | C(0) interleaved with A(1) |
            # softmax(1) | C(1)
            for blk in range(nblk_b):
                stage_a_block(0, blk)
            softmax_b(0)
            for blk in range(nblk_b):
                stage_c_block(0, blk)
                if bl > 1:
                    stage_a_block(1, blk)
            stage_c_finish(0)
            if bl > 1:
                softmax_b(1)
                for blk in range(nblk_b):
                    stage_c_block(1, blk)
                stage_c_finish(1)

    nc.compile()
    return nc


def make_in_maps(s_t_hat, encoder_outputs, encoder_feature, enc_padding_mask,
                 coverage, W_dec, b_dec, W_c, v, n_cores=NCORES, bl=BL, s_len=S):
    tpb = s_len // P
    nt = bl * tpb
    rows = bl * s_len
    b_total = n_cores * bl

    ef_full = np.ascontiguousarray(encoder_feature, np.float32).reshape(
        b_total, s_len, H)
    eo_full = np.ascontiguousarray(encoder_outputs, np.float32)
    s_full = np.ascontiguousarray(s_t_hat, np.float32)
    mask_full = np.ascontiguousarray(enc_padding_mask, np.float32)
    cov_full = np.ascontiguousarray(coverage, np.float32)

    wdec = np.ascontiguousarray(W_dec, np.float32)
    wc = np.ascontiguousarray(W_c, np.float32).reshape(1, H)
    bdecc = np.ascontiguousarray(
        np.asarray(b_dec, np.float32).reshape(KH, P).T)          # [P, KH]
    vrep = np.ascontiguousarray(
        np.broadcast_to(np.asarray(v, np.float32)[None, :], (P, H)))
    ident = np.eye(P, dtype=np.float32)

    in_maps = []
    for i in range(n_cores):
        b0 = i * bl
        maskc = np.ascontiguousarray(
            mask_full[b0:b0 + bl].reshape(bl, tpb, P)
            .transpose(2, 0, 1).reshape(P, nt))
        in_maps.append({
            "ef": np.ascontiguousarray(ef_full[b0:b0 + bl].reshape(rows, H)),
            "eo": np.ascontiguousarray(eo_full[b0:b0 + bl].reshape(rows, H)),
            "srep": np.ascontiguousarray(
                np.broadcast_to(s_full[b0:b0 + bl, None, :], (bl, P, H))),
            "r2lc": np.ascontiguousarray(np.stack([
                np.ones(rows, np.float32),
                cov_full[b0:b0 + bl].reshape(-1)])),
            "maskc": maskc,
            "wdec": wdec,
            "wc": wc,
            "bdecc": bdecc,
            "vrep": vrep,
            "ident": ident,
        })
    return in_maps


def _gather(results, n_cores=NCORES):
    ct = np.concatenate([results[i]["ct_out"] for i in range(n_cores)], 0)
    attn = np.concatenate([results[i]["attn_out"] for i in range(n_cores)], 0)
    scores = np.concatenate([results[i]["scores_out"] for i in range(n_cores)], 0)
    return ct, attn, attn, scores


_NC_CACHE = {}


def _get_program():
    if "nc" not in _NC_CACHE:
        _NC_CACHE["nc"] = build_program()
    return _NC_CACHE["nc"]


def run(trace=False, tmpdir=None, **inputs):
    nc = _get_program()
    in_maps = make_in_maps(**inputs)
    res = run_bass_kernel_spmd(nc, in_maps, list(range(NCORES)), trace=trace,
                               tmpdir=tmpdir)
    return _gather(res.results), res


def kernel(**inputs):
    out, _ = run(trace=False, **inputs)
    return out
